# revision 1
# baseline (speedup 1.0000x reference)
"""KMeans vq_codebook step on 8 NeuronCores (Trainium2, Bass/Tile).

Data-parallel over N: each core gets x/y shard [8192, 512]/[8192], centers
replicated. Per core, per 128-point tile:
  xs   = block-swizzled x load (DMA)         -> DVE 32x32 stream-transpose
  s    = 2*x@centers.T - ||c||^2             (PE f32r, rank-1 seeds -c2)
  s_sb = copy PSUM->SBUF                     (ACT)
  m8   = row max8(s_sb)(DVE);  mask = (s_sb == m) bf16 (DVE)
  counts^T += onehot(y).T @ mask             (PE bf16, PSUM accumulate)
  x2 partial via ACT square+accum (order-free: host sums partitions)
Host: sum partial counts/losses across cores, max/sum for acc.
"""
import sys

sys.path.insert(0, "/opt/trn_rl_repo")

import numpy as np

import concourse.bass as bass
import concourse.mybir as mybir
from concourse import bacc
from concourse.bass import ds, ts
from concourse.bass_utils import run_bass_kernel_spmd
from concourse.masks import make_identity
from concourse.tile import TileContext

dt = mybir.dt
F32 = dt.float32
F32R = dt.float32r
BF16 = dt.bfloat16
I32 = dt.int32
AF = mybir.ActivationFunctionType
ALU = mybir.AluOpType

N, D, K, NCLS, NCORES = 65536, 512, 1024, 10, 8
NSH = N // NCORES          # 8192 points per core
PT = NSH // 128            # 64 point-tiles per core
DC = D // 128              # 4 contraction chunks
KH = K // 512              # 2 free-dim halves

USE_F32R = True            # measured on HW: loss 3e-7, acc 3e-4 rel err


def _build(use_f32r: bool):
    mmdt = F32R if use_f32r else F32
    nc = bacc.Bacc(None, target_bir_lowering=False, debug=False)
    x_in = nc.dram_tensor("x", [NSH, D], F32, kind="ExternalInput")
    c_in = nc.dram_tensor("centers", [K, D], F32, kind="ExternalInput")
    y_in = nc.dram_tensor("y", [NSH], I32, kind="ExternalInput")
    counts_out = nc.dram_tensor("counts", [NCLS, K], F32, kind="ExternalOutput")
    loss_out = nc.dram_tensor("loss", [128, 2], F32, kind="ExternalOutput")
    scr = nc.dram_tensor("scr", [K // 128, 128], F32)  # c2 col->row bounce

    with TileContext(nc) as tc:
        with (
            tc.tile_pool(name="persist", bufs=1) as pp,
            tc.tile_pool(name="work", bufs=4) as wp,
            tc.tile_pool(name="psA", bufs=2, space="PSUM") as psA,   # s tiles
            tc.tile_pool(name="psB", bufs=2, space="PSUM") as psB,   # prep/warm
            tc.tile_pool(name="psH", bufs=1, space="PSUM") as psH,   # histogram
        ):
            ident = pp.tile([128, 128], F32)
            make_identity(nc, ident[:])

            # ---- prep: centers -> 2*centers.T (f32r), c2 row, y one-hot aids
            cT2 = pp.tile([128, DC, K], mmdt)       # [d-part, dc, k] = 2*c[k,d]
            c2cols = pp.tile([128, K // 128], F32)
            sq = pp.tile([128, D], F32)
            for kc in range(K // 128):
                ct = wp.tile([128, D], F32, tag="ct")
                nc.sync.dma_start(out=ct[:], in_=c_in[ts(kc, 128), :])
                nc.scalar.activation(sq[:], ct[:], AF.Square,
                                     accum_out=c2cols[:, kc:kc + 1])
                for dc in range(DC):
                    ptr = psB.tile([128, 128], F32, tag="ptr")
                    nc.tensor.transpose(ptr[:], ct[:, ts(dc, 128)], ident[:])
                    nc.scalar.mul(cT2[:, dc, ts(kc, 128)], ptr[:], 2.0)
            # c2 columns -> one 1024-wide row (via DRAM bounce), f32r for rank-1
            nc.sync.dma_start(out=scr[:, :].rearrange("k p -> p k"), in_=c2cols[:])
            c2row_f = pp.tile([1, K], F32)
            nc.sync.dma_start(out=c2row_f[:], in_=scr[:, :].rearrange("k p -> () (k p)"))
            c2full = pp.tile([128, K], F32)
            nc.gpsimd.partition_broadcast(c2full[:], c2row_f[0:1, :], 128)

            iota_i = pp.tile([128, 16], I32)
            nc.gpsimd.iota(iota_i[:], pattern=[[1, 16]], base=0, channel_multiplier=0)
            iota_f = pp.tile([128, 16], F32)
            nc.vector.tensor_copy(iota_f[:], iota_i[:])
            ycol_i = pp.tile([128, PT], I32)
            nc.sync.dma_start(out=ycol_i[:], in_=y_in[:].rearrange("(t p) -> p t", p=128))
            ycol = pp.tile([128, PT], F32)
            nc.vector.tensor_copy(ycol[:], ycol_i[:])

            x2buf = pp.tile([128, PT], F32)
            m8buf = pp.tile([128, PT * 8], F32)
            hist = psH.tile([NCLS, K], F32)

            # ---- PE warmup: ~4us of tiny matmuls right before the main GEMM
            # stream so the HAM clock-gate opens (cold K=4/8 halves PE clock).
            wt_f = pp.tile([128, 128], F32)
            nc.vector.memset(wt_f[:], 0.0)
            wt = wt_f[:].bitcast(BF16)[:, 0:128]
            wps = psB.tile([128, 512], F32, tag="ptr")
            for _ in range(40):
                nc.tensor.matmul(wps[:, 0:128], wt, wt, start=True, stop=True,
                                 skip_group_check=True)

            # ---- main loop over 64 point-tiles
            for t in range(PT):
                xt = wp.tile([128, D], F32, tag="xt")
                nc.scalar.dma_start(out=xt[:], in_=x_in[ts(t, 128), :])
                sqx = wp.tile([128, D], F32, tag="sqx")
                nc.scalar.activation(sqx[:], xt[:], AF.Square,
                                     accum_out=x2buf[:, t:t + 1])
                xT = wp.tile([128, DC, 128], mmdt, tag="xT")
                for dc in range(DC):
                    ptr = psB.tile([128, 128], F32, tag="ptr")
                    nc.tensor.transpose(ptr[:], xt[:, ts(dc, 128)], ident[:])
                    nc.scalar.copy(xT[:, dc, :], ptr[:])
                ps = psA.tile([128, K], F32, tag="ps")
                for kh in range(KH):
                    for dc in range(DC):
                        nc.tensor.matmul(ps[:, ds(kh * 512, 512)], xT[:, dc, :],
                                         cT2[:, dc, ds(kh * 512, 512)],
                                         start=(dc == 0), stop=(dc == DC - 1),
                                         skip_group_check=True)
                s_sb = wp.tile([128, K], F32, tag="s_sb")
                nc.vector.scalar_tensor_tensor(
                    out=s_sb[:], in0=ps[:], scalar=0.0,
                    in1=c2full[:],
                    op0=ALU.add, op1=ALU.subtract)
                nc.vector.max(m8buf[:, ts(t, 8)], s_sb[:])
                maskt = wp.tile([128, K], BF16, tag="mask")
                nc.vector.tensor_scalar(out=maskt[:], in0=s_sb[:],
                                        scalar1=m8buf[:, t * 8:t * 8 + 1],
                                        scalar2=None, op0=ALU.is_equal)
                oht = wp.tile([128, 16], BF16, tag="oht")
                nc.vector.tensor_scalar(out=oht[:], in0=iota_f[:],
                                        scalar1=ycol[:, t:t + 1],
                                        scalar2=None, op0=ALU.is_equal)
                for kh in range(KH):
                    nc.tensor.matmul(hist[:, ds(kh * 512, 512)], oht[:, 0:NCLS],
                                     maskt[:, ds(kh * 512, 512)],
                                     start=(t == 0), stop=(t == PT - 1),
                                     skip_group_check=True)

            # ---- tail: loss partials + counts to DRAM
            lossb = pp.tile([128, 2], F32)
            nc.vector.tensor_reduce(lossb[:, 0:1], x2buf[:], axis=mybir.AxisListType.X,
                                    op=ALU.add)
            m8v = m8buf[:].rearrange("p (t e) -> p t e", e=8)[:, :, 0:1]
            nc.vector.tensor_reduce(lossb[:, 1:2], m8v, axis=mybir.AxisListType.XY,
                                    op=ALU.add)
            nc.sync.dma_start(out=loss_out[:], in_=lossb[:])
            csb = pp.tile([NCLS, K], F32)
            nc.scalar.copy(csb[:], hist[:])
            nc.sync.dma_start(out=counts_out[:], in_=csb[:])

    nc.finalize()
    return nc


_NC_CACHE: dict = {}


def _get_nc(use_f32r: bool = USE_F32R):
    if use_f32r not in _NC_CACHE:
        _NC_CACHE[use_f32r] = _build(use_f32r)
    return _NC_CACHE[use_f32r]


def kernel(x, centers, y, _trace=False, _use_f32r=USE_F32R):
    x = np.ascontiguousarray(np.asarray(x, dtype=np.float32))
    centers = np.ascontiguousarray(np.asarray(centers, dtype=np.float32))
    y = np.ascontiguousarray(np.asarray(y, dtype=np.int32))
    nc = _get_nc(_use_f32r)
    in_maps = [
        {"x": x[c * NSH:(c + 1) * NSH], "centers": centers,
         "y": y[c * NSH:(c + 1) * NSH]}
        for c in range(NCORES)
    ]
    res = run_bass_kernel_spmd(nc, in_maps, core_ids=list(range(NCORES)),
                               trace=_trace)
    counts = np.zeros((NCLS, K), np.float64)
    loss = 0.0
    for r in res.results:
        counts += r["counts"].astype(np.float64)
        loss += (r["loss"][:, 0].astype(np.float64)
                 - r["loss"][:, 1].astype(np.float64)).sum()
    correct = counts.max(axis=0).sum()
    acc = np.float32(correct / N)
    out = (np.float32(loss), acc)
    if _trace:
        return out, res
    return out



# revision 2
# speedup vs baseline: 1.0493x; 1.0493x over previous
"""KMeans vq_codebook step on 8 NeuronCores (Trainium2, Bass/Tile).

Data-parallel over N: each core gets x/y shard [8192, 512]/[8192], centers
replicated. s' = x@c.T - c2/2 is computed per 128-point tile in fp8e4m3 with
DoubleRow matmuls (2x contraction per pass); the -c2/2 bias is seeded into
PSUM by an f32r identity matmul (exact); row max via DVE max8 from PSUM; the
argmax one-hot mask = Exp(SC*(s'-m)) on ACT (winner == 1.0 exactly); counts
histogram accumulates onehot(y).T @ mask in PSUM via fp8 DoubleRow matmuls
over tile PAIRS. Host: loss = sum(x2) - 2*sum(m), counts all-reduce + argmax.
"""
import sys

sys.path.insert(0, "/opt/trn_rl_repo")

import numpy as np

import concourse.bass as bass
import concourse.mybir as mybir
from concourse import bacc
from concourse.bass import ds, ts
from concourse.bass_utils import run_bass_kernel_spmd
from concourse.masks import make_identity
from concourse.tile import TileContext

dt = mybir.dt
F32 = dt.float32
F32R = dt.float32r
BF16 = dt.bfloat16
FP8 = dt.float8e4
I32 = dt.int32
AF = mybir.ActivationFunctionType
ALU = mybir.AluOpType
DR = mybir.MatmulPerfMode.DoubleRow

N, D, K, NCLS, NCORES = 65536, 512, 1024, 10, 8
NSH = N // NCORES          # 8192 points per core
PT = NSH // 128            # 64 point-tiles per core
NPAIR = PT // 2            # 32 tile pairs
DC = D // 128              # 4 contraction chunks
SC = 1024.0                # exp sharpness for the argmax mask


def _build():
    nc = bacc.Bacc(None, target_bir_lowering=False, debug=False)
    x_in = nc.dram_tensor("x", [NSH, D], F32, kind="ExternalInput")
    c_in = nc.dram_tensor("centers", [K, D], F32, kind="ExternalInput")
    y_in = nc.dram_tensor("y", [NSH], I32, kind="ExternalInput")
    counts_out = nc.dram_tensor("counts", [16, K], F32, kind="ExternalOutput")
    loss_out = nc.dram_tensor("loss", [128, 2], F32, kind="ExternalOutput")
    scr = nc.dram_tensor("scr", [K // 128, 128], F32)  # c2 col->row bounce

    with TileContext(nc) as tc:
        with (
            tc.tile_pool(name="persist", bufs=1) as pp,
            tc.tile_pool(name="work", bufs=3) as wp,
            tc.tile_pool(name="psA", bufs=2, space="PSUM") as psA,   # s tiles
            tc.tile_pool(name="psB", bufs=2, space="PSUM") as psB,   # transpose staging
            tc.tile_pool(name="psH", bufs=1, space="PSUM") as psH,   # histogram
        ):
            ident = pp.tile([128, 128], F32)
            make_identity(nc, ident[:])
            identr = pp.tile([128, 128], F32R)
            nc.vector.tensor_copy(identr[:], ident[:])

            # ---- prep: centers -> cT2 fp8 [d,dc,k]; c2 -> -c2/2 row, replicated
            cT2 = pp.tile([128, DC, K], FP8)
            c2cols = pp.tile([128, K // 128], F32)
            sqc = pp.tile([128, D], F32)
            for kc in range(K // 128):
                ct = wp.tile([128, D], F32, tag="ct")
                nc.sync.dma_start(out=ct[:], in_=c_in[ts(kc, 128), :])
                nc.scalar.activation(sqc[:], ct[:], AF.Square,
                                     accum_out=c2cols[:, kc:kc + 1])
                stg = psB.tile([128, D], F32, tag="tp")
                for dc in range(DC):
                    nc.tensor.matmul(stg[:, ts(dc, 128)], ct[:, ts(dc, 128)],
                                     ident[:], is_transpose=True,
                                     start=(dc == 0), stop=(dc == DC - 1),
                                     skip_group_check=True)
                nc.vector.tensor_copy(
                    cT2[:, :, ts(kc, 128)],
                    stg[:].rearrange("p (a b) -> p a b", a=DC))
            # c2 columns -> one 1024-wide row (via DRAM bounce), then -c2/2
            # replicated across partitions for the identity seed matmul.
            nc.sync.dma_start(out=scr[:, :].rearrange("k p -> p k"), in_=c2cols[:])
            c2row_f = pp.tile([1, K], F32)
            nc.sync.dma_start(out=c2row_f[:], in_=scr[:, :].rearrange("k p -> () (k p)"))
            c2full = pp.tile([128, K], F32)
            nc.gpsimd.partition_broadcast(c2full[:], c2row_f[0:1, :], 128)
            negc2x = pp.tile([128, K], F32R)
            nc.vector.tensor_scalar_mul(negc2x[:], c2full[:], -0.5)

            iota_i = pp.tile([128, 16], I32)
            nc.gpsimd.iota(iota_i[:], pattern=[[1, 16]], base=0, channel_multiplier=0)
            iota_f = pp.tile([128, 16], F32)
            nc.vector.tensor_copy(iota_f[:], iota_i[:])
            ycol_i = pp.tile([128, PT], I32)
            nc.sync.dma_start(out=ycol_i[:], in_=y_in[:].rearrange("(t p) -> p t", p=128))
            ycol = pp.tile([128, PT], F32)
            nc.vector.tensor_copy(ycol[:], ycol_i[:])

            x2buf = pp.tile([128, NPAIR], F32)
            m8buf = pp.tile([128, PT * 8], F32)
            negm = pp.tile([128, PT], F32)
            hist = psH.tile([16, K], F32)

            # ---- PE warmup: ~4us of tiny matmuls right before the main GEMM
            # stream so the HAM clock-gate opens (cold K=4/8 halves PE clock).
            wt_f = pp.tile([128, 128], F32)
            nc.vector.memset(wt_f[:], 0.0)
            wt = wt_f[:].bitcast(BF16)[:, 0:128]
            wps = psB.tile([128, D], F32, tag="tp")
            for _ in range(40):
                nc.tensor.matmul(wps[:, 0:128], wt, wt, start=True, stop=True,
                                 skip_group_check=True)

            # ---- main loop over 32 pairs of 128-point tiles
            prev = None  # (ohtp, mp) of previous pair, hist delayed one pair
            for pr in range(NPAIR):
                xpair = wp.tile([128, 2, D], F32, tag="xp")
                nc.sync.dma_start(
                    out=xpair[:],
                    in_=x_in[ds(pr * 256, 256), :].rearrange(
                        "(two p) d -> p two d", two=2))
                sqscr = wp.tile([128, 2 * D], F32, tag="sq")
                nc.scalar.activation(sqscr[:],
                                     xpair[:].rearrange("p two d -> p (two d)"),
                                     AF.Square, accum_out=x2buf[:, pr:pr + 1])
                mp = wp.tile([128, 2, K], FP8, tag="mp")
                ohtp = wp.tile([128, 2, 16], FP8, tag="oh")
                for i in range(2):
                    t = 2 * pr + i
                    stg = psB.tile([128, D], F32, tag="tp")
                    for dc in range(DC):
                        nc.tensor.matmul(stg[:, ts(dc, 128)],
                                         xpair[:, i, ts(dc, 128)], ident[:],
                                         is_transpose=True,
                                         start=(dc == 0), stop=(dc == DC - 1),
                                         skip_group_check=True)
                    xT = wp.tile([128, DC, 128], FP8, tag="xT")
                    nc.vector.tensor_copy(
                        xT[:].rearrange("p a b -> p (a b)"), stg[:])
                    ps = psA.tile([128, K], F32, tag="ps")
                    for kh in range(2):
                        nc.tensor.matmul(ps[:, ds(kh * 512, 512)], identr[:],
                                         negc2x[:, ds(kh * 512, 512)],
                                         start=True, stop=False,
                                         skip_group_check=True)
                    for g in range(2):
                        for kh in range(2):
                            nc.tensor.matmul(
                                ps[:, ds(kh * 512, 512)],
                                xT[:, ds(2 * g, 2), :],
                                cT2[:, ds(2 * g, 2), ds(kh * 512, 512)],
                                start=False, stop=(g == 1),
                                perf_mode=DR, skip_group_check=True)
                    nc.vector.max(m8buf[:, ts(t, 8)], ps[:])
                    nc.vector.tensor_scalar_mul(negm[:, t:t + 1],
                                                m8buf[:, t * 8:t * 8 + 1], -SC)
                    nc.scalar.activation(mp[:, i, :], ps[:], AF.Exp,
                                         bias=negm[:, t:t + 1], scale=SC)
                    nc.vector.tensor_scalar(out=ohtp[:, i, :], in0=iota_f[:],
                                            scalar1=ycol[:, t:t + 1],
                                            scalar2=None, op0=ALU.is_equal)
                if prev is not None:
                    pohtp, pmp, ppr = prev
                    for kh in range(2):
                        nc.tensor.matmul(hist[:, ds(kh * 512, 512)],
                                         pohtp[:, :, :],
                                         pmp[:, :, ds(kh * 512, 512)],
                                         start=(ppr == 0), stop=False,
                                         perf_mode=DR, skip_group_check=True)
                prev = (ohtp, mp, pr)
            pohtp, pmp, ppr = prev
            for kh in range(2):
                nc.tensor.matmul(hist[:, ds(kh * 512, 512)], pohtp[:, :, :],
                                 pmp[:, :, ds(kh * 512, 512)],
                                 start=False, stop=True,
                                 perf_mode=DR, skip_group_check=True)

            # ---- tail: loss partials + counts to DRAM
            lossb = pp.tile([128, 2], F32)
            nc.vector.tensor_reduce(lossb[:, 0:1], x2buf[:], axis=mybir.AxisListType.X,
                                    op=ALU.add)
            m8v = m8buf[:].rearrange("p (t e) -> p t e", e=8)[:, :, 0:1]
            nc.vector.tensor_reduce(lossb[:, 1:2], m8v, axis=mybir.AxisListType.XY,
                                    op=ALU.add)
            nc.sync.dma_start(out=loss_out[:], in_=lossb[:])
            csb = pp.tile([16, K], F32)
            nc.scalar.copy(csb[:], hist[:])
            nc.sync.dma_start(out=counts_out[:], in_=csb[:])

    nc.finalize()
    return nc


_NC_CACHE: dict = {}


def _get_nc():
    if "nc" not in _NC_CACHE:
        _NC_CACHE["nc"] = _build()
    return _NC_CACHE["nc"]


def kernel(x, centers, y, _trace=False):
    x = np.ascontiguousarray(np.asarray(x, dtype=np.float32))
    centers = np.ascontiguousarray(np.asarray(centers, dtype=np.float32))
    y = np.ascontiguousarray(np.asarray(y, dtype=np.int32))
    nc = _get_nc()
    in_maps = [
        {"x": x[c * NSH:(c + 1) * NSH], "centers": centers,
         "y": y[c * NSH:(c + 1) * NSH]}
        for c in range(NCORES)
    ]
    res = run_bass_kernel_spmd(nc, in_maps, core_ids=list(range(NCORES)),
                               trace=_trace)
    counts = np.zeros((16, K), np.float64)
    loss = 0.0
    for r in res.results:
        counts += r["counts"].astype(np.float64)
        loss += (r["loss"][:, 0].astype(np.float64)
                 - 2.0 * r["loss"][:, 1].astype(np.float64)).sum()
    correct = counts[:NCLS].max(axis=0).sum()
    acc = np.float32(correct / N)
    out = (np.float32(loss), acc)
    if _trace:
        return out, res
    return out
